# revision 1
# baseline (speedup 1.0000x reference)
"""MLGRU cell on 8 Trainium2 NeuronCores.

Reference math (per batch element b, all matmuls contract over d=2048):
    f = sigmoid(x @ tern(wf).T + bf)
    c = silu   (x @ tern(wc).T + bc)
    h = f * h_prev + (1 - f) * c
    g = sigmoid(x @ tern(wg).T + bg)
    o = (g * h) @ tern(wo).T + bo
    return (o, h)

Sharding: data-parallel over batch, one batch element per core (B=8, 8 cores,
no collectives).

Device layout: features-on-partitions ([o, t] tiles) everywhere, so neither
activations nor weights ever need an on-device transpose:
  - stage A matmul: lhsT = w_t[d_chunk, o_chunk]  (K=d on partitions),
                    rhs  = x_t[d_chunk, t_chunk]  -> psum [o, t]
  - gating is elementwise in [o, t]; u = g*h is produced directly in the
    [feature, t] layout that stage B needs as its rhs (K = feature dim).
Weights are ternarized + transposed + cast to bf16 host-side (weight
pre-formatting); x is transposed + cast bf16 host-side. Matmuls run in bf16
(ternary weights are exact in bf16) with fp32 PSUM accumulation.
"""

import sys

if "/opt/trn_rl_repo" not in sys.path:
    sys.path.insert(0, "/opt/trn_rl_repo")

import numpy as np
import ml_dtypes

import concourse.bass as bass
import concourse.mybir as mybir
import concourse.tile as tile
from concourse import bacc
from concourse.bass_utils import run_bass_kernel_spmd

BF16 = ml_dtypes.bfloat16
B, S, D = 8, 2048, 2048
P = 128
KO = D // P   # 16 contraction chunks
NJ = D // P   # 16 output-feature blocks
TB = 512      # token-block (matmul free dim / PSUM bank)
NTB = S // TB  # 4
THRESH = np.float32(0.33)

F32 = mybir.dt.float32
BF = mybir.dt.bfloat16
AF = mybir.ActivationFunctionType

_CACHE = {}


def build_nc():
    """Per-core Bass program. Inputs are pre-formatted host-side (see kernel)."""
    nc = bacc.Bacc("TRN2", target_bir_lowering=False, debug=False, num_devices=8)

    xt_d = nc.dram_tensor("xt", (P, KO, S), BF, kind="ExternalInput")
    hpt_d = nc.dram_tensor("hpt", (D, S), F32, kind="ExternalInput")
    w_d = {
        k: nc.dram_tensor(f"w4{k}", (NJ, P, KO, P), BF, kind="ExternalInput")
        for k in ("f", "c", "g", "o")
    }
    b_d = {
        k: nc.dram_tensor(f"b4{k}", (P, NJ), F32, kind="ExternalInput")
        for k in ("f", "c", "g", "o")
    }
    ht_d = nc.dram_tensor("ht", (D, S), F32, kind="ExternalOutput")
    ot_d = nc.dram_tensor("ot", (D, S), F32, kind="ExternalOutput")

    with tile.TileContext(nc) as tc:
        with (
            tc.tile_pool(name="xp", bufs=1) as xp,
            tc.tile_pool(name="up", bufs=1) as up,
            tc.tile_pool(name="wp", bufs=4) as wp,
            tc.tile_pool(name="wop", bufs=2) as wop,
            tc.tile_pool(name="hpp", bufs=3) as hpp,
            tc.tile_pool(name="actp", bufs=3) as actp,
            tc.tile_pool(name="tmpp", bufs=3) as tmpp,
            tc.tile_pool(name="outp", bufs=3) as outp,
            tc.tile_pool(name="biasp", bufs=1) as biasp,
            tc.tile_pool(name="psA", bufs=6, space="PSUM") as psA,
            tc.tile_pool(name="psB", bufs=2, space="PSUM") as psB,
        ):
            # first x block + first gate weights up front so the PE can
            # start after ~3.5MB of DMA instead of all of x (8MB) + weights
            xts = []
            t = xp.tile([P, KO, TB], BF, tag="x0", name="x0")
            nc.sync.dma_start(t[:], xt_d[:, :, 0:TB])
            xts.append(t)
            w0 = {}
            for k in ("f", "c", "g"):
                w = wp.tile([P, KO, P], BF, tag="wgate", name=f"w0{k}")
                nc.sync.dma_start(w[:], w_d[k][0])
                w0[k] = w
            for tb in range(1, NTB):
                t = xp.tile([P, KO, TB], BF, tag=f"x{tb}", name=f"x{tb}")
                nc.sync.dma_start(t[:], xt_d[:, :, tb * TB:(tb + 1) * TB])
                xts.append(t)
            bt = {}
            for k in ("f", "c", "g", "o"):
                t = biasp.tile([P, NJ], F32, tag=f"bias_{k}")
                nc.sync.dma_start(t[:], b_d[k][:])
                bt[k] = t
            uts = [up.tile([P, KO, TB], BF, tag=f"u{tb}", name=f"u{tb}")
                   for tb in range(NTB)]

            # ---- stage A: f/c/g projections + gating, fills U ----
            for j in range(NJ):
                if j == 0:
                    wt = w0
                else:
                    wt = {}
                    for k in ("f", "c", "g"):
                        w = wp.tile([P, KO, P], BF, tag="wgate")
                        nc.sync.dma_start(w[:], w_d[k][j])
                        wt[k] = w
                for tb in range(NTB):
                    ts_ = slice(tb * TB, (tb + 1) * TB)
                    ps = {}
                    for k in ("f", "c", "g"):
                        p = psA.tile([P, TB], F32, tag="psA")
                        for ko in range(KO):
                            nc.tensor.matmul(
                                p[:], wt[k][:, ko, :], xts[tb][:, ko, :],
                                start=(ko == 0), stop=(ko == KO - 1),
                            )
                        ps[k] = p
                    fs = actp.tile([P, TB], F32, tag="fs")
                    nc.scalar.activation(fs[:], ps["f"][:], AF.Sigmoid,
                                         bias=bt["f"][:, j:j + 1])
                    cs = actp.tile([P, TB], F32, tag="cs")
                    nc.scalar.activation(cs[:], ps["c"][:], AF.Silu,
                                         bias=bt["c"][:, j:j + 1])
                    gs = actp.tile([P, TB], F32, tag="gs")
                    nc.scalar.activation(gs[:], ps["g"][:], AF.Sigmoid,
                                         bias=bt["g"][:, j:j + 1])
                    hp = hpp.tile([P, TB], F32, tag="hp")
                    nc.sync.dma_start(hp[:], hpt_d[j * P:(j + 1) * P, ts_])
                    # h = c + f*(h_prev - c)
                    d1 = tmpp.tile([P, TB], F32, tag="d1")
                    nc.vector.tensor_sub(d1[:], hp[:], cs[:])
                    d2 = tmpp.tile([P, TB], F32, tag="d2")
                    nc.vector.tensor_mul(d2[:], fs[:], d1[:])
                    hs = outp.tile([P, TB], F32, tag="hs")
                    nc.vector.tensor_add(hs[:], d2[:], cs[:])
                    nc.sync.dma_start(ht_d[j * P:(j + 1) * P, ts_], hs[:])
                    nc.vector.tensor_mul(uts[tb][:, j, :], gs[:], hs[:])

            # ---- stage B: o = U.T-contraction with wo ----
            for j in range(NJ):
                w = wop.tile([P, KO, P], BF, tag="wo")
                nc.sync.dma_start(w[:], w_d["o"][j])
                for tb in range(NTB):
                    ts_ = slice(tb * TB, (tb + 1) * TB)
                    p = psB.tile([P, TB], F32, tag="psB")
                    for ko in range(KO):
                        nc.tensor.matmul(
                            p[:], w[:, ko, :], uts[tb][:, ko, :],
                            start=(ko == 0), stop=(ko == KO - 1),
                        )
                    os_ = outp.tile([P, TB], F32, tag="os")
                    nc.vector.tensor_scalar(os_[:], p[:], bt["o"][:, j:j + 1],
                                            None, mybir.AluOpType.add)
                    nc.sync.dma_start(ot_d[j * P:(j + 1) * P, ts_], os_[:])

    nc.compile()
    return nc


def _ternary(w):
    return np.where(np.abs(w) < THRESH, np.float32(0.0),
                    np.sign(w)).astype(np.float32)


def _fmt_w(w):
    # w: [o, d] fp32 -> w4[j, kp, ko, oi] = tern(w)[j*128+oi, ko*128+kp], bf16
    wt = _ternary(np.asarray(w)).T  # [d, o]
    w4 = wt.reshape(KO, P, NJ, P).transpose(2, 1, 0, 3)
    return np.ascontiguousarray(w4).astype(BF16)


def _fmt_x(xb):
    # xb: [t, d] fp32 -> xt[kp, ko, t] = xb[t, ko*128+kp], bf16
    xt = np.asarray(xb).T.reshape(KO, P, S).transpose(1, 0, 2)
    return np.ascontiguousarray(xt).astype(BF16)


def kernel(x, h_prev, wf, bf, wc, bc, wg, bg, wo, bo):
    if "nc" not in _CACHE:
        _CACHE["nc"] = build_nc()
    nc = _CACHE["nc"]

    w4 = {"f": _fmt_w(wf), "c": _fmt_w(wc), "g": _fmt_w(wg), "o": _fmt_w(wo)}
    b4 = {
        k: np.ascontiguousarray(np.asarray(v, np.float32).reshape(NJ, P).T)
        for k, v in (("f", bf), ("c", bc), ("g", bg), ("o", bo))
    }
    in_maps = []
    for b in range(B):
        m = {"xt": _fmt_x(x[b]),
             "hpt": np.ascontiguousarray(np.asarray(h_prev[b]).T)}
        for k in ("f", "c", "g", "o"):
            m[f"w4{k}"] = w4[k]
            m[f"b4{k}"] = b4[k]
        in_maps.append(m)

    res = run_bass_kernel_spmd(nc, in_maps, core_ids=list(range(B)))

    o = np.empty((B, S, D), np.float32)
    h = np.empty((B, S, D), np.float32)
    for b in range(B):
        o[b] = res.results[b]["ot"].T
        h[b] = res.results[b]["ht"].T
    return o, h

